# revision 36
# baseline (speedup 1.0000x reference)
"""GATr-style geometric-algebra transformer block on 8 Trainium2 NeuronCores.

Strategy
--------
Host (numpy): fuse `blade` into every equi_linear weight so the device only
sees plain matmuls; pre-select the inner-product coordinates for q/k
(so qs/ks are computed directly); fold the 1/sqrt(256) attention scale into
the q weights; pre-transpose/pack everything into the exact SBUF layouts the
kernel wants.

Sharding: 8 cores = 4 batches x 2 query-halves. Every core runs the SAME
program (SPMD); the per-core "query half" is realized by rotating the token
axis of that core's input so its queries are always tokens [0, 1024).
k/v are computed over the full (rotated) sequence - softmax/attention are
permutation invariant over keys.

On-device dataflow (channel-major activations [feature_part, token_free]):
  xT -> enter proj -> hT -> equi-norm (masked mean-square via mask-matmul,
  rsqrt, gpsimd partition-broadcast) -> hn -> ks/v projections
  -> per head: qs proj, scores = qsT.T @ ksT (PSUM, fp32),
     softmax (DVE row-max + ACT exp with per-partition bias + accum sum),
     PE-transpose of bf16 probs -> attnT, attn@v in bf16 (fp32 PSUM accum),
     1/sumexp folded into the PSUM->SBUF copy of the per-head output,
     out_proj accumulated into acc (which was initialized with the residual),
  -> final projection -> outT.
"""

import sys

import numpy as np

for _p in ("/opt/trn_rl_repo", "/root/.axon_site/_ro/trn_rl_repo"):
    if _p not in sys.path:
        sys.path.insert(0, _p)

import ml_dtypes  # noqa: E402

import concourse.bacc as bacc  # noqa: E402
import concourse.tile as tile  # noqa: E402
from concourse import mybir  # noqa: E402
from concourse.bass_utils import run_bass_kernel_spmd  # noqa: E402

F32 = mybir.dt.float32
F16 = mybir.dt.float16
BF16 = mybir.dt.bfloat16
AX = mybir.AxisListType.X
AF = mybir.ActivationFunctionType

INNER = np.array([0, 2, 3, 4, 8, 9, 10, 14])
B, S, C_IN, MV = 4, 2048, 3, 16
HID, NH = 32, 8
NCORES = 8
SLAB = S // 2  # queries per core

# Set by test.py to collect an NTFF profile.
TRACE = False
LAST_RESULTS = None


# --------------------------------------------------------------------------
# Device program
# --------------------------------------------------------------------------

def _emit(tc):
    nc = tc.nc

    xT_d = nc.declare_dram_parameter("xT", [48, S], F32, isOutput=False)
    w_enter_d = nc.declare_dram_parameter("w_enter", [48, 512], F32, isOutput=False)
    w_qsh_d = nc.declare_dram_parameter("w_qsh", [128, 4, 2048], F16, isOutput=False)
    w_qsl_d = nc.declare_dram_parameter("w_qsl", [128, 4, 2048], F16, isOutput=False)
    w_ksh_d = nc.declare_dram_parameter("w_ksh", [128, 4, 256], F16, isOutput=False)
    w_ksl_d = nc.declare_dram_parameter("w_ksl", [128, 4, 256], F16, isOutput=False)
    w_v_d = nc.declare_dram_parameter("w_v", [128, 4, 512], BF16, isOutput=False)
    w_out_d = nc.declare_dram_parameter("w_out", [128, 32, 512], BF16, isOutput=False)
    w_fin_d = nc.declare_dram_parameter("w_final", [128, 4, 512], F32, isOutput=False)
    mask_d = nc.declare_dram_parameter("mask", [128, 4], F32, isOutput=False)
    idf_d = nc.declare_dram_parameter("ident_f", [128, 128], F32, isOutput=False)
    idh_d = nc.declare_dram_parameter("ident_h", [128, 128], BF16, isOutput=False)
    outT_d = nc.declare_dram_parameter("outT", [512, SLAB], F32, isOutput=True)

    from contextlib import ExitStack

    with ExitStack() as ctx:
        psum = ctx.enter_context(tc.tile_pool(name="ps", bufs=1, space="PSUM"))
        pp = ctx.enter_context(tc.tile_pool(name="persist", bufs=1))
        wp = ctx.enter_context(tc.tile_pool(name="wbig", bufs=2))

        # ---- persistent tiles -------------------------------------------
        # hn kept only as fp16 hi/lo splits (+ bf16 for the v path):
        # every consumer matmul runs split-fp16 or bf16.
        hh = pp.tile([128, 4, S], F16, name="hh")
        hl = pp.tile([128, 4, S], F16, name="hl")
        hn_bf = pp.tile([128, 4, S], BF16, name="hn_bf")
        # ks stored as fp16 hi/lo split (scores run as 3 fp16 products)
        kh = pp.tile([128, 2, S], F16, name="kh")
        kl = pp.tile([128, 2, S], F16, name="kl")
        v_tok = pp.tile([128, 16, 512], BF16, name="v_tok")
        acc = pp.tile([128, 4, SLAB], F32, name="acc")
        w_ksh = pp.tile([128, 4, 256], F16, name="w_ksh")
        w_ksl = pp.tile([128, 4, 256], F16, name="w_ksl")
        mask = pp.tile([128, 4], F32, name="mask")
        idf = pp.tile([128, 128], F32, name="idf")
        idh = pp.tile([128, 128], BF16, name="idh")

        nc.sync.dma_start(out=w_ksh[:], in_=w_ksh_d[:, :, :])
        nc.sync.dma_start(out=w_ksl[:], in_=w_ksl_d[:, :, :])
        nc.sync.dma_start(out=mask[:], in_=mask_d[:, :])
        nc.sync.dma_start(out=idf[:], in_=idf_d[:, :])
        nc.sync.dma_start(out=idh[:], in_=idh_d[:, :])

        # ================= phase A: enter, norm, k/v =====================
        with tc.tile_pool(name="pA", bufs=1) as pA:
            xT = pA.tile([48, S], F32, name="xT")
            w_enter = pA.tile([48, 512], F32, name="w_enter")
            hT = pA.tile([128, 4, S], F32, name="hT")
            nc.sync.dma_start(out=xT[:], in_=xT_d[:, :])
            nc.sync.dma_start(out=w_enter[:], in_=w_enter_d[:, :])

            # enter projection: hT[mt] = W_enter[:, mt].T @ xT
            for mt in range(4):
                for c in range(4):
                    ps = psum.tile([128, 512], F32, tag="av", name="ps_ent")
                    nc.tensor.matmul(
                        ps[:],
                        w_enter[0:48, mt * 128:(mt + 1) * 128],
                        xT[0:48, c * 512:(c + 1) * 512],
                        start=True, stop=True,
                    )
                    nc.scalar.copy(hT[:, mt, c * 512:(c + 1) * 512], ps[:])
                # residual lives in acc
                nc.scalar.copy(acc[:, mt, :], hT[:, mt, 0:SLAB])

            # equi-norm: ip = masked mean of squares over channels (PE reduce)
            ipc = [psum.tile([1, 512], F32, tag="sc", bufs=4, name="ipc")
                   for _ in range(4)]
            for kt in range(4):
                hsq = pA.tile([128, S], F32, tag="hsq", bufs=2, name="hsq")
                nc.scalar.activation(hsq[:], hT[:, kt, :], AF.Square)
                for c in range(4):
                    nc.tensor.matmul(
                        ipc[c][0:1, :],
                        mask[:, kt:kt + 1],
                        hsq[:, c * 512:(c + 1) * 512],
                        start=(kt == 0), stop=(kt == 3),
                    )
            sq_s = pA.tile([1, S], F32, name="sq_s")
            r_tok = pA.tile([1, S], F32, name="r_tok")
            rb = pA.tile([128, S], F32, name="rb")
            for c in range(4):
                cs = slice(c * 512, (c + 1) * 512)
                nc.scalar.activation(sq_s[0:1, cs], ipc[c][0:1, :], AF.Sqrt)
                nc.vector.reciprocal(r_tok[0:1, cs], sq_s[0:1, cs])
                nc.gpsimd.partition_broadcast(rb[:, cs], r_tok[0:1, cs])
                for kt in range(4):
                    hnc = pA.tile([128, 512], F32, tag="hnc", bufs=3,
                                  name="hnc")
                    nc.vector.tensor_mul(hnc[:], hT[:, kt, cs], rb[:, cs])
                    nc.scalar.copy(hh[:, kt, cs], hnc[:])
                    nc.vector.tensor_sub(hl[:, kt, cs], hnc[:], hh[:, kt, cs])
                    nc.scalar.copy(hn_bf[:, kt, cs], hnc[:])

            # ks projection (3-term fp16 split products, fp32 PSUM)
            kprods = [(w_ksh, hh), (w_ksl, hh), (w_ksh, hl)]
            for mt in range(2):
                for c in range(4):
                    ps = psum.tile([128, 512], F32, tag="av", name="ps_ks")
                    for p in range(3):
                        lw, lh = kprods[p]
                        for kt in range(4):
                            nc.tensor.matmul(
                                ps[:],
                                lw[:, kt, mt * 128:(mt + 1) * 128],
                                lh[:, kt, c * 512:(c + 1) * 512],
                                start=(p == 0 and kt == 0),
                                stop=(p == 2 and kt == 3),
                            )
                    nc.scalar.copy(kh[:, mt, c * 512:(c + 1) * 512], ps[:])
                    nc.vector.tensor_sub(kl[:, mt, c * 512:(c + 1) * 512],
                                         ps[:], kh[:, mt, c * 512:(c + 1) * 512])

            # v projection weights (the matmuls run as initial pipeline
            # pieces interleaved with the first head's softmax chains)
            w_v = wp.tile([128, 4, 512], BF16, tag="wbig", name="w_v")
            nc.sync.dma_start(out=w_v[:], in_=w_v_d[:, :, :])

        def v_piece(j):
            def piece():
                for tt in range(4 * j, 4 * j + 4):
                    ps = psum.tile([128, 512], F32, tag="av", name="ps_v")
                    for kt in range(4):
                        nc.tensor.matmul(
                            ps[:],
                            hn_bf[:, kt, tt * 128:(tt + 1) * 128],
                            w_v[:, kt, :],
                            start=(kt == 0), stop=(kt == 3),
                        )
                    nc.scalar.copy(v_tok[:, tt, :], ps[:])
            return piece

        # ================= phase B: attention over 8 heads ===============
        # Software pipeline: for each (h, qg), the tail stages (last q-tile's
        # transposes, attn@v, out_proj) are deferred until after the NEXT
        # (h, qg)'s scores+softmax have been emitted, so the exp latency of
        # the last q-tile always hides under independent PE work.
        with tc.tile_pool(name="pB", bufs=1) as pB:
            pending = [[v_piece(j) for j in range(4)]]

            def transposes(attn_q, attnT, qt):
                # qt-major: 16 transposed blocks of attn[qt] -> attnT
                for j in range(4):
                    tr = psum.tile([128, 512], BF16, tag="tr", name="tr")
                    for k in range(4):
                        tt = j * 4 + k
                        nc.tensor.transpose(
                            tr[:, k * 128:(k + 1) * 128],
                            attn_q[qt][:, tt * 128:(tt + 1) * 128],
                            idh[:],
                        )
                    nc.scalar.copy(
                        attnT[j][:, :, qt * 128:(qt + 1) * 128],
                        tr[:].rearrange("p (k q) -> p k q", k=4),
                    )

            def make_finisher(attn_q, attnT, sexp, w_out, qg):
                # Four pieces, fired one per q-tile of the NEXT scores block,
                # so PE always has independent work while softmax chains run.
                state = {}

                def p0():
                    transposes(attn_q, attnT, 3)
                    # 1/sumexp -> free axis -> broadcast over partitions
                    st = psum.tile([1, 512], F32, tag="tr", name="st")
                    for qt in range(4):
                        nc.tensor.transpose(
                            st[0:1, qt * 128:(qt + 1) * 128],
                            sexp[:, qt:qt + 1], idf[:],
                        )
                    rqT = pB.tile([1, 512], F32, tag="rqT", bufs=2, name="rqT")
                    nc.vector.reciprocal(rqT[:], st[0:1, :])
                    rb_q = pB.tile([128, 512], F32, tag="rbq", bufs=2, name="rb_q")
                    nc.gpsimd.partition_broadcast(rb_q[:], rqT[0:1, :])
                    state["rb_q"] = rb_q
                    state["oT"] = pB.tile([128, 4, 512], BF16, tag="oT",
                                          bufs=2, name="oT")

                def av(mt):
                    avp = psum.tile([128, 512], F32, tag="av", name="avp")
                    for tt in range(16):
                        nc.tensor.matmul(
                            avp[:],
                            v_tok[:, tt, mt * 128:(mt + 1) * 128],
                            attnT[tt // 4][:, tt % 4, :],
                            start=(tt == 0), stop=(tt == 15),
                        )
                    nc.vector.tensor_mul(state["oT"][:, mt, :], avp[:],
                                         state["rb_q"][:])

                def p1():
                    av(0)
                    av(1)

                def p2():
                    av(2)
                    av(3)

                def p3():
                    for mt2 in range(4):
                        opp = psum.tile([128, 512], F32, tag="av", name="opp")
                        for kt in range(4):
                            nc.tensor.matmul(
                                opp[:],
                                w_out[:, kt, mt2 * 128:(mt2 + 1) * 128],
                                state["oT"][:, kt, :],
                                start=(kt == 0), stop=(kt == 3),
                            )
                        sl = acc[:, mt2, qg * 512:(qg + 1) * 512]
                        nc.vector.tensor_add(sl, sl, opp[:])
                return [p0, p1, p2, p3]

            for h in range(NH):
                w_qsh = pB.tile([128, 4, 256], F16, tag="wqsh", bufs=2,
                                name="w_qsh")
                w_qsl = pB.tile([128, 4, 256], F16, tag="wqsl", bufs=2,
                                name="w_qsl")
                nc.sync.dma_start(out=w_qsh[:],
                                  in_=w_qsh_d[:, :, h * 256:(h + 1) * 256])
                nc.sync.dma_start(out=w_qsl[:],
                                  in_=w_qsl_d[:, :, h * 256:(h + 1) * 256])
                w_out = wp.tile([128, 4, 512], BF16, tag="wbig", name="w_out")
                nc.sync.dma_start(out=w_out[:], in_=w_out_d[:, 4 * h:4 * h + 4, :])

                # qs projection (3-term fp16 split products, fp32 PSUM)
                qh = pB.tile([128, 2, SLAB], F16, tag="qh", bufs=2, name="qh")
                ql = pB.tile([128, 2, SLAB], F16, tag="ql", bufs=2, name="ql")
                qprods = [(w_qsh, hh), (w_qsl, hh), (w_qsh, hl)]
                for mt in range(2):
                    for c in range(SLAB // 512):
                        ps = psum.tile([128, 512], F32, tag="tr", name="ps_qs")
                        for p in range(3):
                            lw, lh = qprods[p]
                            for kt in range(4):
                                nc.tensor.matmul(
                                    ps[:],
                                    lw[:, kt, mt * 128:(mt + 1) * 128],
                                    lh[:, kt, c * 512:(c + 1) * 512],
                                    start=(p == 0 and kt == 0),
                                    stop=(p == 2 and kt == 3),
                                )
                        cs = slice(c * 512, (c + 1) * 512)
                        nc.scalar.copy(qh[:, mt, cs], ps[:])
                        nc.vector.tensor_sub(ql[:, mt, cs], ps[:], qh[:, mt, cs])

                for qg in range(SLAB // 512):
                    attn_q = []
                    sexp = pB.tile([128, 4], F32, tag="sexp", bufs=2, name="sexp")
                    attnT = []
                    for j in range(4):
                        attnT.append(pB.tile([128, 4, 512], BF16, tag="attnT",
                                             bufs=8, name="attnT"))

                    for qt in range(4):
                        qtg = qg * 4 + qt
                        qsl = slice(qtg * 128, (qtg + 1) * 128)
                        sc_c = [psum.tile([128, 512], F32, tag="sc", bufs=4,
                                          name="scc") for _ in range(4)]
                        cmax = pB.tile([128, 4], F32, tag="cmax", bufs=4,
                                       name="cmax")
                        # scores = qh*kh + ql*kh + qh*kl  (fp16 split, fp32 acc)
                        prods = [(qh, kh), (ql, kh), (qh, kl)]
                        steps = [(p, kt2) for p in range(3) for kt2 in range(2)]
                        if qt < 3:
                            # step-major: stationary operand reused across chunks
                            for si, (p, kt2) in enumerate(steps):
                                lq, lk = prods[p]
                                for t4 in range(4):
                                    nc.tensor.matmul(
                                        sc_c[t4][:],
                                        lq[:, kt2, qsl],
                                        lk[:, kt2, t4 * 512:(t4 + 1) * 512],
                                        start=(si == 0), stop=(si == 5),
                                    )
                                    if si == 5:
                                        nc.vector.reduce_max(
                                            cmax[:, t4:t4 + 1], sc_c[t4][:],
                                            axis=AX)
                        else:
                            # last q-tile: chunk-major so each chunk's max can
                            # start while later chunks are still multiplying —
                            # this tile's softmax tail is the exposed one
                            for t4 in range(4):
                                for si, (p, kt2) in enumerate(steps):
                                    lq, lk = prods[p]
                                    nc.tensor.matmul(
                                        sc_c[t4][:],
                                        lq[:, kt2, qsl],
                                        lk[:, kt2, t4 * 512:(t4 + 1) * 512],
                                        start=(si == 0), stop=(si == 5),
                                    )
                                nc.vector.reduce_max(
                                    cmax[:, t4:t4 + 1], sc_c[t4][:], axis=AX)
                        negmax = pB.tile([128, 1], F32, tag="negmax", bufs=4,
                                         name="negmax")
                        nc.vector.reduce_max(negmax[:], cmax[:], axis=AX,
                                             negate=True)
                        attn = pB.tile([128, S], BF16, tag="attn", bufs=4, name="attn")
                        seh = pB.tile([128, 4], F32, tag="seh", bufs=4, name="seh")
                        for t4 in range(4):
                            nc.scalar.activation(
                                attn[:, t4 * 512:(t4 + 1) * 512],
                                sc_c[t4][:], AF.Exp,
                                bias=negmax[:], scale=1.0,
                                accum_out=seh[:, t4:t4 + 1],
                            )
                        nc.vector.reduce_sum(sexp[:, qt:qt + 1], seh[:], axis=AX)
                        attn_q.append(attn)
                        if pending[0] is not None:
                            pending[0][qt]()
                        if qt >= 1:
                            transposes(attn_q, attnT, qt - 1)

                    pending[0] = make_finisher(attn_q, attnT, sexp, w_out, qg)

            for piece in pending[0]:
                piece()

        # ================= phase C: final projection =====================
        with tc.tile_pool(name="pC", bufs=1) as pC:
            w_fin = wp.tile([128, 4, 512], F32, tag="wbig", name="w_fin")
            nc.sync.dma_start(out=w_fin[:], in_=w_fin_d[:, :, :])
            for mt in range(4):
                for c in range(SLAB // 512):
                    fp = psum.tile([128, 512], F32, tag="av", name="fp")
                    for kt in range(4):
                        nc.tensor.matmul(
                            fp[:],
                            w_fin[:, kt, mt * 128:(mt + 1) * 128],
                            acc[:, kt, c * 512:(c + 1) * 512],
                            start=(kt == 0), stop=(kt == 3),
                        )
                    outc = pC.tile([128, 512], F32, tag="outc", bufs=2, name="outc")
                    nc.scalar.copy(outc[:], fp[:])
                    nc.sync.dma_start(
                        out=outT_d[mt * 128:(mt + 1) * 128,
                                   c * 512:(c + 1) * 512],
                        in_=outc[:],
                    )


_NC_CACHE = None


def _get_nc():
    global _NC_CACHE
    if _NC_CACHE is None:
        nc = bacc.Bacc("TRN2", debug=False, num_devices=NCORES)
        with tile.TileContext(nc) as tc:
            _emit(tc)
        nc.compile()
        _NC_CACHE = nc
    return _NC_CACHE


# --------------------------------------------------------------------------
# Host-side packing
# --------------------------------------------------------------------------

def _fuse(w, blade):
    # einsum('jib,bxy->jixy', w, blade) -> mat[(i,x), (j,y)]
    wb = np.einsum("jib,bxy->jixy", w, blade)
    j, i = w.shape[0], w.shape[1]
    return np.ascontiguousarray(wb.transpose(1, 2, 0, 3)).reshape(i * 16, j * 16)


def _to_kt(m, kparts):
    # [K, F] -> [128, K//128, F] partition-major packing
    k, f = m.shape
    assert k == kparts * 128
    return np.ascontiguousarray(m.reshape(kparts, 128, f).transpose(1, 0, 2))


def _pack_weights(blade, w_enter, w_q, w_k, w_v, w_out, w_final):
    f32 = np.float32
    W_enter = _fuse(w_enter, blade).astype(f32)  # [48, 512]

    Wq = _fuse(w_q, blade)  # [512, 4096], col (j, y), j = d*8 + h
    Wq = Wq.reshape(512, 32, 8, 16)[:, :, :, INNER]   # [c, d, h, yi]
    Wq = Wq.transpose(0, 2, 1, 3).reshape(512, 8, 256)  # [c, h, (d,yi)]
    Wqs = (Wq.reshape(512, 2048) / 16.0).astype(f32)   # fold 1/sqrt(256)

    Wk = _fuse(w_k, blade)  # [512, 512], col (d, y)
    Wks = Wk.reshape(512, 32, 16)[:, :, INNER].reshape(512, 256).astype(f32)

    Wv = _fuse(w_v, blade).astype(f32)       # [512, 512]
    Wo = _fuse(w_out, blade).astype(f32)     # [4096, 512], rows (h, d, x)
    Wf = _fuse(w_final, blade).astype(f32)   # [512, 512]

    maskv = np.zeros(512, f32)
    for d in range(32):
        maskv[d * 16 + INNER] = 1.0 / 32.0
    mask = np.ascontiguousarray(maskv.reshape(4, 128).T)

    Wqs_kt = _to_kt(Wqs, 4)
    Wqsh = Wqs_kt.astype(np.float16)
    Wqsl = (Wqs_kt - Wqsh.astype(np.float32)).astype(np.float16)
    Wks_kt = _to_kt(Wks, 4)
    Wksh = Wks_kt.astype(np.float16)
    Wksl = (Wks_kt - Wksh.astype(np.float32)).astype(np.float16)
    return {
        "w_enter": W_enter,
        "w_qsh": Wqsh,
        "w_qsl": Wqsl,
        "w_ksh": Wksh,
        "w_ksl": Wksl,
        "w_v": _to_kt(Wv, 4).astype(ml_dtypes.bfloat16),
        "w_out": _to_kt(Wo, 32).astype(ml_dtypes.bfloat16),
        "w_final": _to_kt(Wf, 4),
        "mask": mask,
        "ident_f": np.eye(128, dtype=f32),
        "ident_h": np.eye(128).astype(ml_dtypes.bfloat16),
    }


def kernel(x, blade, w_enter, w_q, w_k, w_v, w_out, w_final):
    global LAST_RESULTS
    x = np.asarray(x, np.float32)
    shared = _pack_weights(
        np.asarray(blade, np.float32), np.asarray(w_enter, np.float32),
        np.asarray(w_q, np.float32), np.asarray(w_k, np.float32),
        np.asarray(w_v, np.float32), np.asarray(w_out, np.float32),
        np.asarray(w_final, np.float32),
    )

    in_maps = []
    for c in range(NCORES):
        b, half = c // 2, c % 2
        xb = x[b].reshape(S, 48)
        xb = np.roll(xb, -SLAB * half, axis=0)
        m = dict(shared)
        m["xT"] = np.ascontiguousarray(xb.T)
        in_maps.append(m)

    nc = _get_nc()
    res = run_bass_kernel_spmd(
        nc, in_maps, core_ids=list(range(NCORES)), trace=TRACE,
    )
    LAST_RESULTS = res

    out = np.empty((B, S, HID, MV), np.float32)
    for c in range(NCORES):
        b, half = c // 2, c % 2
        outT = res.results[c]["outT"]  # [512, 1024]
        out[b, half * SLAB:(half + 1) * SLAB] = (
            outT.T.reshape(SLAB, HID, MV)
        )
    return out


# revision 37
# speedup vs baseline: 1.3867x; 1.3867x over previous
"""GATr-style geometric-algebra transformer block on 8 Trainium2 NeuronCores.

Strategy
--------
Host (numpy): fuse `blade` into every equi_linear weight so the device only
sees plain matmuls; pre-select the inner-product coordinates for q/k
(so qs/ks are computed directly); fold the 1/sqrt(256) attention scale into
the q weights; pre-transpose/pack everything into the exact SBUF layouts the
kernel wants.

Sharding: 8 cores = 4 batches x 2 query-halves. Every core runs the SAME
program (SPMD); the per-core "query half" is realized by rotating the token
axis of that core's input so its queries are always tokens [0, 1024).
k/v are computed over the full (rotated) sequence - softmax/attention are
permutation invariant over keys.

On-device dataflow (channel-major activations [feature_part, token_free]):
  xT -> enter proj -> hT -> equi-norm (masked mean-square via mask-matmul,
  rsqrt, gpsimd partition-broadcast) -> hn -> ks/v projections
  -> per head: qs proj, scores = qsT.T @ ksT (PSUM, fp32),
     softmax (DVE row-max + ACT exp with per-partition bias + accum sum),
     PE-transpose of bf16 probs -> attnT, attn@v in bf16 (fp32 PSUM accum),
     1/sumexp folded into the PSUM->SBUF copy of the per-head output,
     out_proj accumulated into acc (which was initialized with the residual),
  -> final projection -> outT.
"""

import sys

import numpy as np

for _p in ("/opt/trn_rl_repo", "/root/.axon_site/_ro/trn_rl_repo"):
    if _p not in sys.path:
        sys.path.insert(0, _p)

import ml_dtypes  # noqa: E402

import concourse.bacc as bacc  # noqa: E402
import concourse.tile as tile  # noqa: E402
from concourse import mybir  # noqa: E402
from concourse.bass_utils import run_bass_kernel_spmd  # noqa: E402

F32 = mybir.dt.float32
F16 = mybir.dt.float16
BF16 = mybir.dt.bfloat16
AX = mybir.AxisListType.X
AF = mybir.ActivationFunctionType

INNER = np.array([0, 2, 3, 4, 8, 9, 10, 14])
B, S, C_IN, MV = 4, 2048, 3, 16
HID, NH = 32, 8
NCORES = 8
SLAB = S // 2  # queries per core

# Set by test.py to collect an NTFF profile.
TRACE = False
LAST_RESULTS = None


# --------------------------------------------------------------------------
# Device program
# --------------------------------------------------------------------------

def _emit(tc):
    nc = tc.nc

    xT_d = nc.declare_dram_parameter("xT", [48, S], F32, isOutput=False)
    w_enter_d = nc.declare_dram_parameter("w_enter", [48, 512], F32, isOutput=False)
    w_qsh_d = nc.declare_dram_parameter("w_qsh", [128, 4, 2048], F16, isOutput=False)
    w_qsl_d = nc.declare_dram_parameter("w_qsl", [128, 4, 2048], F16, isOutput=False)
    w_ksh_d = nc.declare_dram_parameter("w_ksh", [128, 4, 256], F16, isOutput=False)
    w_ksl_d = nc.declare_dram_parameter("w_ksl", [128, 4, 256], F16, isOutput=False)
    w_v_d = nc.declare_dram_parameter("w_v", [128, 4, 512], F16, isOutput=False)
    w_out_d = nc.declare_dram_parameter("w_out", [128, 32, 512], BF16, isOutput=False)
    w_fin_d = nc.declare_dram_parameter("w_final", [128, 4, 512], F32, isOutput=False)
    mask_d = nc.declare_dram_parameter("mask", [128, 4], F32, isOutput=False)
    idf_d = nc.declare_dram_parameter("ident_f", [128, 128], F32, isOutput=False)
    idh_d = nc.declare_dram_parameter("ident_h", [128, 128], BF16, isOutput=False)
    outT_d = nc.declare_dram_parameter("outT", [512, SLAB], F32, isOutput=True)

    from contextlib import ExitStack

    with ExitStack() as ctx:
        psum = ctx.enter_context(tc.tile_pool(name="ps", bufs=1, space="PSUM"))
        pp = ctx.enter_context(tc.tile_pool(name="persist", bufs=1))
        wp = ctx.enter_context(tc.tile_pool(name="wbig", bufs=2))

        # ---- persistent tiles -------------------------------------------
        # hn kept only as fp16 hi/lo splits (+ bf16 for the v path):
        # every consumer matmul runs split-fp16 or bf16.
        hh = pp.tile([128, 4, S], F16, name="hh")
        hl = pp.tile([128, 4, S], F16, name="hl")
        # ks stored as fp16 hi/lo split (scores run as 3 fp16 products)
        kh = pp.tile([128, 2, S], F16, name="kh")
        kl = pp.tile([128, 2, S], F16, name="kl")
        v_tok = pp.tile([128, 16, 512], BF16, name="v_tok")
        acc = pp.tile([128, 4, SLAB], F32, name="acc")
        w_ksh = pp.tile([128, 4, 256], F16, name="w_ksh")
        w_ksl = pp.tile([128, 4, 256], F16, name="w_ksl")
        mask = pp.tile([128, 4], F32, name="mask")
        idf = pp.tile([128, 128], F32, name="idf")
        idh = pp.tile([128, 128], BF16, name="idh")

        nc.sync.dma_start(out=w_ksh[:], in_=w_ksh_d[:, :, :])
        nc.sync.dma_start(out=w_ksl[:], in_=w_ksl_d[:, :, :])
        nc.sync.dma_start(out=mask[:], in_=mask_d[:, :])
        nc.sync.dma_start(out=idf[:], in_=idf_d[:, :])
        nc.sync.dma_start(out=idh[:], in_=idh_d[:, :])

        # ================= phase A: enter, norm, k/v =====================
        with tc.tile_pool(name="pA", bufs=1) as pA:
            xT = pA.tile([48, S], F32, name="xT")
            w_enter = pA.tile([48, 512], F32, name="w_enter")
            hT = pA.tile([128, 4, S], F32, name="hT")
            nc.sync.dma_start(out=xT[:], in_=xT_d[:, :])
            nc.sync.dma_start(out=w_enter[:], in_=w_enter_d[:, :])

            # enter projection: hT[mt] = W_enter[:, mt].T @ xT
            for mt in range(4):
                for c in range(4):
                    ps = psum.tile([128, 512], F32, tag="av", name="ps_ent")
                    nc.tensor.matmul(
                        ps[:],
                        w_enter[0:48, mt * 128:(mt + 1) * 128],
                        xT[0:48, c * 512:(c + 1) * 512],
                        start=True, stop=True,
                    )
                    nc.scalar.copy(hT[:, mt, c * 512:(c + 1) * 512], ps[:])
                # residual lives in acc
                nc.scalar.copy(acc[:, mt, :], hT[:, mt, 0:SLAB])

            # equi-norm: ip = masked mean of squares over channels (PE reduce)
            ipc = [psum.tile([1, 512], F32, tag="sc", bufs=4, name="ipc")
                   for _ in range(4)]
            for kt in range(4):
                hsq = pA.tile([128, S], F32, tag="hsq", bufs=2, name="hsq")
                nc.scalar.activation(hsq[:], hT[:, kt, :], AF.Square)
                for c in range(4):
                    nc.tensor.matmul(
                        ipc[c][0:1, :],
                        mask[:, kt:kt + 1],
                        hsq[:, c * 512:(c + 1) * 512],
                        start=(kt == 0), stop=(kt == 3),
                    )
            sq_s = pA.tile([1, S], F32, name="sq_s")
            r_tok = pA.tile([1, S], F32, name="r_tok")
            rb = pA.tile([128, S], F32, name="rb")
            for c in range(4):
                cs = slice(c * 512, (c + 1) * 512)
                nc.scalar.activation(sq_s[0:1, cs], ipc[c][0:1, :], AF.Sqrt)
                nc.vector.reciprocal(r_tok[0:1, cs], sq_s[0:1, cs])
                nc.gpsimd.partition_broadcast(rb[:, cs], r_tok[0:1, cs])
                for kt in range(4):
                    hnc = pA.tile([128, 512], F32, tag="hnc", bufs=3,
                                  name="hnc")
                    nc.vector.tensor_mul(hnc[:], hT[:, kt, cs], rb[:, cs])
                    nc.scalar.copy(hh[:, kt, cs], hnc[:])
                    nc.vector.tensor_sub(hl[:, kt, cs], hnc[:], hh[:, kt, cs])

            # ks projection (3-term fp16 split products, fp32 PSUM)
            kprods = [(w_ksh, hh), (w_ksl, hh), (w_ksh, hl)]
            for mt in range(2):
                for c in range(4):
                    ps = psum.tile([128, 512], F32, tag="av", name="ps_ks")
                    for p in range(3):
                        lw, lh = kprods[p]
                        for kt in range(4):
                            nc.tensor.matmul(
                                ps[:],
                                lw[:, kt, mt * 128:(mt + 1) * 128],
                                lh[:, kt, c * 512:(c + 1) * 512],
                                start=(p == 0 and kt == 0),
                                stop=(p == 2 and kt == 3),
                            )
                    nc.scalar.copy(kh[:, mt, c * 512:(c + 1) * 512], ps[:])
                    nc.vector.tensor_sub(kl[:, mt, c * 512:(c + 1) * 512],
                                         ps[:], kh[:, mt, c * 512:(c + 1) * 512])

            # v projection weights (the matmuls run as initial pipeline
            # pieces interleaved with the first head's softmax chains)
            w_v = wp.tile([128, 4, 512], F16, tag="wbig", name="w_v")
            nc.sync.dma_start(out=w_v[:], in_=w_v_d[:, :, :])

        def v_piece(j):
            def piece():
                for tt in range(4 * j, 4 * j + 4):
                    ps = psum.tile([128, 512], F32, tag="av", name="ps_v")
                    for kt in range(4):
                        nc.tensor.matmul(
                            ps[:],
                            hh[:, kt, tt * 128:(tt + 1) * 128],
                            w_v[:, kt, :],
                            start=(kt == 0), stop=(kt == 3),
                        )
                    nc.scalar.copy(v_tok[:, tt, :], ps[:])
            return piece

        # ================= phase B: attention over 8 heads ===============
        # Software pipeline: for each (h, qg), the tail stages (last q-tile's
        # transposes, attn@v, out_proj) are deferred until after the NEXT
        # (h, qg)'s scores+softmax have been emitted, so the exp latency of
        # the last q-tile always hides under independent PE work.
        with tc.tile_pool(name="pB", bufs=1) as pB:
            pending = [[v_piece(j) for j in range(4)]]

            def transposes(attn_q, attnT, qt):
                # qt-major: 16 transposed blocks of attn[qt] -> attnT
                for j in range(4):
                    tr = psum.tile([128, 512], BF16, tag="tr", name="tr")
                    for k in range(4):
                        tt = j * 4 + k
                        nc.tensor.transpose(
                            tr[:, k * 128:(k + 1) * 128],
                            attn_q[qt][:, tt * 128:(tt + 1) * 128],
                            idh[:],
                        )
                    nc.scalar.copy(
                        attnT[j][:, :, qt * 128:(qt + 1) * 128],
                        tr[:].rearrange("p (k q) -> p k q", k=4),
                    )

            def make_finisher(attn_q, attnT, sexp, w_out, qg):
                # Four pieces, fired one per q-tile of the NEXT scores block,
                # so PE always has independent work while softmax chains run.
                state = {}

                def p0():
                    transposes(attn_q, attnT, 3)
                    # 1/sumexp -> free axis -> broadcast over partitions
                    st = psum.tile([1, 512], F32, tag="tr", name="st")
                    for qt in range(4):
                        nc.tensor.transpose(
                            st[0:1, qt * 128:(qt + 1) * 128],
                            sexp[:, qt:qt + 1], idf[:],
                        )
                    rqT = pB.tile([1, 512], F32, tag="rqT", bufs=2, name="rqT")
                    nc.vector.reciprocal(rqT[:], st[0:1, :])
                    rb_q = pB.tile([128, 512], F32, tag="rbq", bufs=2, name="rb_q")
                    nc.gpsimd.partition_broadcast(rb_q[:], rqT[0:1, :])
                    state["rb_q"] = rb_q
                    state["oT"] = pB.tile([128, 4, 512], BF16, tag="oT",
                                          bufs=2, name="oT")

                def av(mt):
                    avp = psum.tile([128, 512], F32, tag="av", name="avp")
                    for tt in range(16):
                        nc.tensor.matmul(
                            avp[:],
                            v_tok[:, tt, mt * 128:(mt + 1) * 128],
                            attnT[tt // 4][:, tt % 4, :],
                            start=(tt == 0), stop=(tt == 15),
                        )
                    nc.vector.tensor_mul(state["oT"][:, mt, :], avp[:],
                                         state["rb_q"][:])

                def p1():
                    av(0)
                    av(1)

                def p2():
                    av(2)
                    av(3)

                def p3():
                    for mt2 in range(4):
                        opp = psum.tile([128, 512], F32, tag="av", name="opp")
                        for kt in range(4):
                            nc.tensor.matmul(
                                opp[:],
                                w_out[:, kt, mt2 * 128:(mt2 + 1) * 128],
                                state["oT"][:, kt, :],
                                start=(kt == 0), stop=(kt == 3),
                            )
                        sl = acc[:, mt2, qg * 512:(qg + 1) * 512]
                        nc.vector.tensor_add(sl, sl, opp[:])
                return [p0, p1, p2, p3]

            for h in range(NH):
                w_qsh = pB.tile([128, 4, 256], F16, tag="wqsh", bufs=2,
                                name="w_qsh")
                w_qsl = pB.tile([128, 4, 256], F16, tag="wqsl", bufs=2,
                                name="w_qsl")
                nc.sync.dma_start(out=w_qsh[:],
                                  in_=w_qsh_d[:, :, h * 256:(h + 1) * 256])
                nc.sync.dma_start(out=w_qsl[:],
                                  in_=w_qsl_d[:, :, h * 256:(h + 1) * 256])
                w_out = wp.tile([128, 4, 512], BF16, tag="wbig", name="w_out")
                nc.sync.dma_start(out=w_out[:], in_=w_out_d[:, 4 * h:4 * h + 4, :])

                # qs projection (3-term fp16 split products, fp32 PSUM)
                qh = pB.tile([128, 2, SLAB], F16, tag="qh", bufs=2, name="qh")
                ql = pB.tile([128, 2, SLAB], F16, tag="ql", bufs=2, name="ql")
                qprods = [(w_qsh, hh), (w_qsl, hh), (w_qsh, hl)]
                for mt in range(2):
                    for c in range(SLAB // 512):
                        ps = psum.tile([128, 512], F32, tag="tr", name="ps_qs")
                        for p in range(3):
                            lw, lh = qprods[p]
                            for kt in range(4):
                                nc.tensor.matmul(
                                    ps[:],
                                    lw[:, kt, mt * 128:(mt + 1) * 128],
                                    lh[:, kt, c * 512:(c + 1) * 512],
                                    start=(p == 0 and kt == 0),
                                    stop=(p == 2 and kt == 3),
                                )
                        cs = slice(c * 512, (c + 1) * 512)
                        nc.scalar.copy(qh[:, mt, cs], ps[:])
                        nc.vector.tensor_sub(ql[:, mt, cs], ps[:], qh[:, mt, cs])

                for qg in range(SLAB // 512):
                    attn_q = []
                    sexp = pB.tile([128, 4], F32, tag="sexp", bufs=2, name="sexp")
                    attnT = []
                    for j in range(4):
                        attnT.append(pB.tile([128, 4, 512], BF16, tag="attnT",
                                             bufs=8, name="attnT"))

                    for qt in range(4):
                        qtg = qg * 4 + qt
                        qsl = slice(qtg * 128, (qtg + 1) * 128)
                        sc_c = [psum.tile([128, 512], F32, tag="sc", bufs=4,
                                          name="scc") for _ in range(4)]
                        cmax = pB.tile([128, 4], F32, tag="cmax", bufs=4,
                                       name="cmax")
                        # scores = qh*kh + ql*kh + qh*kl  (fp16 split, fp32 acc)
                        prods = [(qh, kh), (ql, kh), (qh, kl)]
                        steps = [(p, kt2) for p in range(3) for kt2 in range(2)]
                        if qt < 3:
                            # step-major: stationary operand reused across chunks
                            for si, (p, kt2) in enumerate(steps):
                                lq, lk = prods[p]
                                for t4 in range(4):
                                    nc.tensor.matmul(
                                        sc_c[t4][:],
                                        lq[:, kt2, qsl],
                                        lk[:, kt2, t4 * 512:(t4 + 1) * 512],
                                        start=(si == 0), stop=(si == 5),
                                    )
                                    if si == 5:
                                        nc.vector.reduce_max(
                                            cmax[:, t4:t4 + 1], sc_c[t4][:],
                                            axis=AX)
                        else:
                            # last q-tile: chunk-major so each chunk's max can
                            # start while later chunks are still multiplying —
                            # this tile's softmax tail is the exposed one
                            for t4 in range(4):
                                for si, (p, kt2) in enumerate(steps):
                                    lq, lk = prods[p]
                                    nc.tensor.matmul(
                                        sc_c[t4][:],
                                        lq[:, kt2, qsl],
                                        lk[:, kt2, t4 * 512:(t4 + 1) * 512],
                                        start=(si == 0), stop=(si == 5),
                                    )
                                nc.vector.reduce_max(
                                    cmax[:, t4:t4 + 1], sc_c[t4][:], axis=AX)
                        negmax = pB.tile([128, 1], F32, tag="negmax", bufs=4,
                                         name="negmax")
                        nc.vector.reduce_max(negmax[:], cmax[:], axis=AX,
                                             negate=True)
                        attn = pB.tile([128, S], BF16, tag="attn", bufs=5, name="attn")
                        seh = pB.tile([128, 4], F32, tag="seh", bufs=4, name="seh")
                        for t4 in range(4):
                            nc.scalar.activation(
                                attn[:, t4 * 512:(t4 + 1) * 512],
                                sc_c[t4][:], AF.Exp,
                                bias=negmax[:], scale=1.0,
                                accum_out=seh[:, t4:t4 + 1],
                            )
                        nc.vector.reduce_sum(sexp[:, qt:qt + 1], seh[:], axis=AX)
                        attn_q.append(attn)
                        if pending[0] is not None:
                            pending[0][qt]()
                        if qt >= 1:
                            transposes(attn_q, attnT, qt - 1)

                    pending[0] = make_finisher(attn_q, attnT, sexp, w_out, qg)

            for piece in pending[0]:
                piece()

        # ================= phase C: final projection =====================
        with tc.tile_pool(name="pC", bufs=1) as pC:
            w_fin = wp.tile([128, 4, 512], F32, tag="wbig", name="w_fin")
            nc.sync.dma_start(out=w_fin[:], in_=w_fin_d[:, :, :])
            for mt in range(4):
                for c in range(SLAB // 512):
                    fp = psum.tile([128, 512], F32, tag="av", name="fp")
                    for kt in range(4):
                        nc.tensor.matmul(
                            fp[:],
                            w_fin[:, kt, mt * 128:(mt + 1) * 128],
                            acc[:, kt, c * 512:(c + 1) * 512],
                            start=(kt == 0), stop=(kt == 3),
                        )
                    outc = pC.tile([128, 512], F32, tag="outc", bufs=2, name="outc")
                    nc.scalar.copy(outc[:], fp[:])
                    nc.sync.dma_start(
                        out=outT_d[mt * 128:(mt + 1) * 128,
                                   c * 512:(c + 1) * 512],
                        in_=outc[:],
                    )


_NC_CACHE = None


def _get_nc():
    global _NC_CACHE
    if _NC_CACHE is None:
        nc = bacc.Bacc("TRN2", debug=False, num_devices=NCORES)
        with tile.TileContext(nc) as tc:
            _emit(tc)
        nc.compile()
        _NC_CACHE = nc
    return _NC_CACHE


# --------------------------------------------------------------------------
# Host-side packing
# --------------------------------------------------------------------------

def _fuse(w, blade):
    # einsum('jib,bxy->jixy', w, blade) -> mat[(i,x), (j,y)]
    wb = np.einsum("jib,bxy->jixy", w, blade)
    j, i = w.shape[0], w.shape[1]
    return np.ascontiguousarray(wb.transpose(1, 2, 0, 3)).reshape(i * 16, j * 16)


def _to_kt(m, kparts):
    # [K, F] -> [128, K//128, F] partition-major packing
    k, f = m.shape
    assert k == kparts * 128
    return np.ascontiguousarray(m.reshape(kparts, 128, f).transpose(1, 0, 2))


def _pack_weights(blade, w_enter, w_q, w_k, w_v, w_out, w_final):
    f32 = np.float32
    W_enter = _fuse(w_enter, blade).astype(f32)  # [48, 512]

    Wq = _fuse(w_q, blade)  # [512, 4096], col (j, y), j = d*8 + h
    Wq = Wq.reshape(512, 32, 8, 16)[:, :, :, INNER]   # [c, d, h, yi]
    Wq = Wq.transpose(0, 2, 1, 3).reshape(512, 8, 256)  # [c, h, (d,yi)]
    Wqs = (Wq.reshape(512, 2048) / 16.0).astype(f32)   # fold 1/sqrt(256)

    Wk = _fuse(w_k, blade)  # [512, 512], col (d, y)
    Wks = Wk.reshape(512, 32, 16)[:, :, INNER].reshape(512, 256).astype(f32)

    Wv = _fuse(w_v, blade).astype(f32)       # [512, 512]
    Wo = _fuse(w_out, blade).astype(f32)     # [4096, 512], rows (h, d, x)
    Wf = _fuse(w_final, blade).astype(f32)   # [512, 512]

    maskv = np.zeros(512, f32)
    for d in range(32):
        maskv[d * 16 + INNER] = 1.0 / 32.0
    mask = np.ascontiguousarray(maskv.reshape(4, 128).T)

    Wqs_kt = _to_kt(Wqs, 4)
    Wqsh = Wqs_kt.astype(np.float16)
    Wqsl = (Wqs_kt - Wqsh.astype(np.float32)).astype(np.float16)
    Wks_kt = _to_kt(Wks, 4)
    Wksh = Wks_kt.astype(np.float16)
    Wksl = (Wks_kt - Wksh.astype(np.float32)).astype(np.float16)
    return {
        "w_enter": W_enter,
        "w_qsh": Wqsh,
        "w_qsl": Wqsl,
        "w_ksh": Wksh,
        "w_ksl": Wksl,
        "w_v": _to_kt(Wv, 4).astype(np.float16),
        "w_out": _to_kt(Wo, 32).astype(ml_dtypes.bfloat16),
        "w_final": _to_kt(Wf, 4),
        "mask": mask,
        "ident_f": np.eye(128, dtype=f32),
        "ident_h": np.eye(128).astype(ml_dtypes.bfloat16),
    }


def kernel(x, blade, w_enter, w_q, w_k, w_v, w_out, w_final):
    global LAST_RESULTS
    x = np.asarray(x, np.float32)
    shared = _pack_weights(
        np.asarray(blade, np.float32), np.asarray(w_enter, np.float32),
        np.asarray(w_q, np.float32), np.asarray(w_k, np.float32),
        np.asarray(w_v, np.float32), np.asarray(w_out, np.float32),
        np.asarray(w_final, np.float32),
    )

    in_maps = []
    for c in range(NCORES):
        b, half = c // 2, c % 2
        xb = x[b].reshape(S, 48)
        xb = np.roll(xb, -SLAB * half, axis=0)
        m = dict(shared)
        m["xT"] = np.ascontiguousarray(xb.T)
        in_maps.append(m)

    nc = _get_nc()
    res = run_bass_kernel_spmd(
        nc, in_maps, core_ids=list(range(NCORES)), trace=TRACE,
    )
    LAST_RESULTS = res

    out = np.empty((B, S, HID, MV), np.float32)
    for c in range(NCORES):
        b, half = c // 2, c % 2
        outT = res.results[c]["outT"]  # [512, 1024]
        out[b, half * SLAB:(half + 1) * SLAB] = (
            outT.T.reshape(SLAB, HID, MV)
        )
    return out


# revision 38
# speedup vs baseline: 1.3869x; 1.0001x over previous
"""GATr-style geometric-algebra transformer block on 8 Trainium2 NeuronCores.

Strategy
--------
Host (numpy): fuse `blade` into every equi_linear weight so the device only
sees plain matmuls; pre-select the inner-product coordinates for q/k
(so qs/ks are computed directly); fold the 1/sqrt(256) attention scale into
the q weights; pre-transpose/pack everything into the exact SBUF layouts the
kernel wants.

Sharding: 8 cores = 4 batches x 2 query-halves. Every core runs the SAME
program (SPMD); the per-core "query half" is realized by rotating the token
axis of that core's input so its queries are always tokens [0, 1024).
k/v are computed over the full (rotated) sequence - softmax/attention are
permutation invariant over keys.

On-device dataflow (channel-major activations [feature_part, token_free]):
  xT -> enter proj -> hT -> equi-norm (masked mean-square via mask-matmul,
  rsqrt, gpsimd partition-broadcast) -> hn -> ks/v projections
  -> per head: qs proj, scores = qsT.T @ ksT (PSUM, fp32),
     softmax (DVE row-max + ACT exp with per-partition bias + accum sum),
     PE-transpose of bf16 probs -> attnT, attn@v in bf16 (fp32 PSUM accum),
     1/sumexp folded into the PSUM->SBUF copy of the per-head output,
     out_proj accumulated into acc (which was initialized with the residual),
  -> final projection -> outT.
"""

import sys

import numpy as np

for _p in ("/opt/trn_rl_repo", "/root/.axon_site/_ro/trn_rl_repo"):
    if _p not in sys.path:
        sys.path.insert(0, _p)

import ml_dtypes  # noqa: E402

import concourse.bacc as bacc  # noqa: E402
import concourse.tile as tile  # noqa: E402
from concourse import mybir  # noqa: E402
from concourse.bass_utils import run_bass_kernel_spmd  # noqa: E402

F32 = mybir.dt.float32
F16 = mybir.dt.float16
BF16 = mybir.dt.bfloat16
AX = mybir.AxisListType.X
AF = mybir.ActivationFunctionType

INNER = np.array([0, 2, 3, 4, 8, 9, 10, 14])
B, S, C_IN, MV = 4, 2048, 3, 16
HID, NH = 32, 8
NCORES = 8
SLAB = S // 2  # queries per core

# Set by test.py to collect an NTFF profile.
TRACE = False
LAST_RESULTS = None


# --------------------------------------------------------------------------
# Device program
# --------------------------------------------------------------------------

def _emit(tc):
    nc = tc.nc

    xT_d = nc.declare_dram_parameter("xT", [48, S], F32, isOutput=False)
    w_enter_d = nc.declare_dram_parameter("w_enter", [48, 512], F32, isOutput=False)
    w_qsh_d = nc.declare_dram_parameter("w_qsh", [128, 4, 2048], F16, isOutput=False)
    w_qsl_d = nc.declare_dram_parameter("w_qsl", [128, 4, 2048], F16, isOutput=False)
    w_ksh_d = nc.declare_dram_parameter("w_ksh", [128, 4, 256], F16, isOutput=False)
    w_ksl_d = nc.declare_dram_parameter("w_ksl", [128, 4, 256], F16, isOutput=False)
    w_v_d = nc.declare_dram_parameter("w_v", [128, 4, 512], F16, isOutput=False)
    w_out_d = nc.declare_dram_parameter("w_out", [128, 32, 512], BF16, isOutput=False)
    w_fin_d = nc.declare_dram_parameter("w_final", [128, 4, 512], F32, isOutput=False)
    mask_d = nc.declare_dram_parameter("mask", [128, 4], F32, isOutput=False)
    idf_d = nc.declare_dram_parameter("ident_f", [128, 128], F32, isOutput=False)
    idh_d = nc.declare_dram_parameter("ident_h", [128, 128], BF16, isOutput=False)
    outT_d = nc.declare_dram_parameter("outT", [512, SLAB], F32, isOutput=True)

    from contextlib import ExitStack

    with ExitStack() as ctx:
        psum = ctx.enter_context(tc.tile_pool(name="ps", bufs=1, space="PSUM"))
        pp = ctx.enter_context(tc.tile_pool(name="persist", bufs=1))
        wp = ctx.enter_context(tc.tile_pool(name="wbig", bufs=2))

        # ---- persistent tiles -------------------------------------------
        # hn kept only as fp16 hi/lo splits (+ bf16 for the v path):
        # every consumer matmul runs split-fp16 or bf16.
        hh = pp.tile([128, 4, S], F16, name="hh")
        hl = pp.tile([128, 4, S], F16, name="hl")
        # ks stored as fp16 hi/lo split (scores run as 3 fp16 products)
        kh = pp.tile([128, 2, S], F16, name="kh")
        kl = pp.tile([128, 2, S], F16, name="kl")
        v_tok = pp.tile([128, 16, 512], BF16, name="v_tok")
        acc = pp.tile([128, 4, SLAB], F32, name="acc")
        w_ksh = pp.tile([128, 4, 256], F16, name="w_ksh")
        w_ksl = pp.tile([128, 4, 256], F16, name="w_ksl")
        mask = pp.tile([128, 4], F32, name="mask")
        idf = pp.tile([128, 128], F32, name="idf")
        idh = pp.tile([128, 128], BF16, name="idh")

        nc.sync.dma_start(out=w_ksh[:], in_=w_ksh_d[:, :, :])
        nc.sync.dma_start(out=w_ksl[:], in_=w_ksl_d[:, :, :])
        nc.sync.dma_start(out=mask[:], in_=mask_d[:, :])
        nc.sync.dma_start(out=idf[:], in_=idf_d[:, :])
        nc.sync.dma_start(out=idh[:], in_=idh_d[:, :])

        # ================= phase A: enter, norm, k/v =====================
        with tc.tile_pool(name="pA", bufs=1) as pA:
            xT = pA.tile([48, S], F32, name="xT")
            w_enter = pA.tile([48, 512], F32, name="w_enter")
            hT = pA.tile([128, 4, S], F32, name="hT")
            nc.sync.dma_start(out=xT[:], in_=xT_d[:, :])
            nc.sync.dma_start(out=w_enter[:], in_=w_enter_d[:, :])

            # enter projection: hT[mt] = W_enter[:, mt].T @ xT
            for mt in range(4):
                for c in range(4):
                    ps = psum.tile([128, 512], F32, tag="av", name="ps_ent")
                    nc.tensor.matmul(
                        ps[:],
                        w_enter[0:48, mt * 128:(mt + 1) * 128],
                        xT[0:48, c * 512:(c + 1) * 512],
                        start=True, stop=True,
                    )
                    nc.scalar.copy(hT[:, mt, c * 512:(c + 1) * 512], ps[:])
                # residual lives in acc
                nc.scalar.copy(acc[:, mt, :], hT[:, mt, 0:SLAB])

            # equi-norm: ip = masked mean of squares over channels (PE reduce)
            ipc = [psum.tile([1, 512], F32, tag="sc", bufs=4, name="ipc")
                   for _ in range(4)]
            for kt in range(4):
                hsq = pA.tile([128, S], F32, tag="hsq", bufs=2, name="hsq")
                nc.scalar.activation(hsq[:], hT[:, kt, :], AF.Square)
                for c in range(4):
                    nc.tensor.matmul(
                        ipc[c][0:1, :],
                        mask[:, kt:kt + 1],
                        hsq[:, c * 512:(c + 1) * 512],
                        start=(kt == 0), stop=(kt == 3),
                    )
            sq_s = pA.tile([1, S], F32, name="sq_s")
            r_tok = pA.tile([1, S], F32, name="r_tok")
            rb = pA.tile([128, S], F32, name="rb")
            for c in range(4):
                cs = slice(c * 512, (c + 1) * 512)
                nc.scalar.activation(sq_s[0:1, cs], ipc[c][0:1, :], AF.Sqrt)
                nc.vector.reciprocal(r_tok[0:1, cs], sq_s[0:1, cs])
                nc.gpsimd.partition_broadcast(rb[:, cs], r_tok[0:1, cs])
                for kt in range(4):
                    hnc = pA.tile([128, 512], F32, tag="hnc", bufs=3,
                                  name="hnc")
                    nc.vector.tensor_mul(hnc[:], hT[:, kt, cs], rb[:, cs])
                    nc.scalar.copy(hh[:, kt, cs], hnc[:])
                    nc.vector.tensor_sub(hl[:, kt, cs], hnc[:], hh[:, kt, cs])

            # ks projection (3-term fp16 split products, fp32 PSUM)
            kprods = [(w_ksh, hh), (w_ksl, hh), (w_ksh, hl)]
            for mt in range(2):
                for c in range(4):
                    ps = psum.tile([128, 512], F32, tag="av", name="ps_ks")
                    for p in range(3):
                        lw, lh = kprods[p]
                        for kt in range(4):
                            nc.tensor.matmul(
                                ps[:],
                                lw[:, kt, mt * 128:(mt + 1) * 128],
                                lh[:, kt, c * 512:(c + 1) * 512],
                                start=(p == 0 and kt == 0),
                                stop=(p == 2 and kt == 3),
                            )
                    nc.scalar.copy(kh[:, mt, c * 512:(c + 1) * 512], ps[:])
                    nc.vector.tensor_sub(kl[:, mt, c * 512:(c + 1) * 512],
                                         ps[:], kh[:, mt, c * 512:(c + 1) * 512])

            # v projection weights (the matmuls run as initial pipeline
            # pieces interleaved with the first head's softmax chains)
            w_v = wp.tile([128, 4, 512], F16, tag="wbig", name="w_v")
            nc.sync.dma_start(out=w_v[:], in_=w_v_d[:, :, :])

        def v_piece(j):
            def piece():
                for tt in range(4 * j, 4 * j + 4):
                    ps = psum.tile([128, 512], F32, tag="av", name="ps_v")
                    for kt in range(4):
                        nc.tensor.matmul(
                            ps[:],
                            hh[:, kt, tt * 128:(tt + 1) * 128],
                            w_v[:, kt, :],
                            start=(kt == 0), stop=(kt == 3),
                        )
                    nc.scalar.copy(v_tok[:, tt, :], ps[:])
            return piece

        # ================= phase B: attention over 8 heads ===============
        # Software pipeline: for each (h, qg), the tail stages (last q-tile's
        # transposes, attn@v, out_proj) are deferred until after the NEXT
        # (h, qg)'s scores+softmax have been emitted, so the exp latency of
        # the last q-tile always hides under independent PE work.
        with tc.tile_pool(name="pB", bufs=1) as pB:
            pending = [[v_piece(j) for j in range(4)]]

            def transposes(attn_q, attnT, qt):
                # qt-major: 16 transposed blocks of attn[qt] -> attnT
                for j in range(4):
                    tr = psum.tile([128, 512], BF16, tag="tr", name="tr")
                    for k in range(4):
                        tt = j * 4 + k
                        nc.tensor.transpose(
                            tr[:, k * 128:(k + 1) * 128],
                            attn_q[qt][:, tt * 128:(tt + 1) * 128],
                            idh[:],
                        )
                    nc.scalar.copy(
                        attnT[j][:, :, qt * 128:(qt + 1) * 128],
                        tr[:].rearrange("p (k q) -> p k q", k=4),
                    )

            def make_finisher(attn_q, attnT, sexp, w_out, qg):
                # Four pieces, fired one per q-tile of the NEXT scores block,
                # so PE always has independent work while softmax chains run.
                state = {}

                def p0():
                    transposes(attn_q, attnT, 3)
                    # 1/sumexp -> free axis -> broadcast over partitions
                    st = psum.tile([1, 512], F32, tag="tr", name="st")
                    for qt in range(4):
                        nc.tensor.transpose(
                            st[0:1, qt * 128:(qt + 1) * 128],
                            sexp[:, qt:qt + 1], idf[:],
                        )
                    rqT = pB.tile([1, 512], F32, tag="rqT", bufs=2, name="rqT")
                    nc.vector.reciprocal(rqT[:], st[0:1, :])
                    rb_q = pB.tile([128, 512], F32, tag="rbq", bufs=2, name="rb_q")
                    nc.gpsimd.partition_broadcast(rb_q[:], rqT[0:1, :])
                    state["rb_q"] = rb_q
                    state["oT"] = pB.tile([128, 4, 512], BF16, tag="oT",
                                          bufs=2, name="oT")

                def av(mt):
                    avp = psum.tile([128, 512], F32, tag="av", name="avp")
                    for tt in range(16):
                        nc.tensor.matmul(
                            avp[:],
                            v_tok[:, tt, mt * 128:(mt + 1) * 128],
                            attnT[tt // 4][:, tt % 4, :],
                            start=(tt == 0), stop=(tt == 15),
                        )
                    nc.vector.tensor_mul(state["oT"][:, mt, :], avp[:],
                                         state["rb_q"][:])

                def p1():
                    av(0)
                    av(1)

                def p2():
                    av(2)
                    av(3)

                def p3():
                    for mt2 in range(4):
                        opp = psum.tile([128, 512], F32, tag="av", name="opp")
                        for kt in range(4):
                            nc.tensor.matmul(
                                opp[:],
                                w_out[:, kt, mt2 * 128:(mt2 + 1) * 128],
                                state["oT"][:, kt, :],
                                start=(kt == 0), stop=(kt == 3),
                            )
                        sl = acc[:, mt2, qg * 512:(qg + 1) * 512]
                        nc.vector.tensor_add(sl, sl, opp[:])
                return [p0, p1, p2, p3]

            for h in range(NH):
                w_qsh = pB.tile([128, 4, 256], F16, tag="wqsh", bufs=2,
                                name="w_qsh")
                w_qsl = pB.tile([128, 4, 256], F16, tag="wqsl", bufs=2,
                                name="w_qsl")
                nc.sync.dma_start(out=w_qsh[:],
                                  in_=w_qsh_d[:, :, h * 256:(h + 1) * 256])
                nc.sync.dma_start(out=w_qsl[:],
                                  in_=w_qsl_d[:, :, h * 256:(h + 1) * 256])
                w_out = wp.tile([128, 4, 512], BF16, tag="wbig", name="w_out")
                nc.sync.dma_start(out=w_out[:], in_=w_out_d[:, 4 * h:4 * h + 4, :])

                # qs projection (3-term fp16 split products, fp32 PSUM)
                qh = pB.tile([128, 2, SLAB], F16, tag="qh", bufs=2, name="qh")
                ql = pB.tile([128, 2, SLAB], F16, tag="ql", bufs=2, name="ql")
                qprods = [(w_qsh, hh), (w_qsl, hh), (w_qsh, hl)]
                for mt in range(2):
                    for c in range(SLAB // 512):
                        ps = psum.tile([128, 512], F32, tag="tr", name="ps_qs")
                        for p in range(3):
                            lw, lh = qprods[p]
                            for kt in range(4):
                                nc.tensor.matmul(
                                    ps[:],
                                    lw[:, kt, mt * 128:(mt + 1) * 128],
                                    lh[:, kt, c * 512:(c + 1) * 512],
                                    start=(p == 0 and kt == 0),
                                    stop=(p == 2 and kt == 3),
                                )
                        cs = slice(c * 512, (c + 1) * 512)
                        nc.scalar.copy(qh[:, mt, cs], ps[:])
                        nc.vector.tensor_sub(ql[:, mt, cs], ps[:], qh[:, mt, cs])

                for qg in range(SLAB // 512):
                    attn_q = []
                    sexp = pB.tile([128, 4], F32, tag="sexp", bufs=2, name="sexp")
                    attnT = []
                    for j in range(4):
                        attnT.append(pB.tile([128, 4, 512], BF16, tag="attnT",
                                             bufs=8, name="attnT"))

                    for qt in range(4):
                        qtg = qg * 4 + qt
                        qsl = slice(qtg * 128, (qtg + 1) * 128)
                        sc_c = [psum.tile([128, 512], F32, tag="sc", bufs=4,
                                          name="scc") for _ in range(4)]
                        cmax = pB.tile([128, 4], F32, tag="cmax", bufs=4,
                                       name="cmax")
                        # scores = qh*kh + ql*kh + qh*kl  (fp16 split, fp32 acc)
                        prods = [(qh, kh), (ql, kh), (qh, kl)]
                        steps = [(p, kt2) for p in range(3) for kt2 in range(2)]
                        if qt < 3:
                            # step-major: stationary operand reused across chunks
                            for si, (p, kt2) in enumerate(steps):
                                lq, lk = prods[p]
                                for t4 in range(4):
                                    nc.tensor.matmul(
                                        sc_c[t4][:],
                                        lq[:, kt2, qsl],
                                        lk[:, kt2, t4 * 512:(t4 + 1) * 512],
                                        start=(si == 0), stop=(si == 5),
                                    )
                                    if si == 5:
                                        nc.vector.reduce_max(
                                            cmax[:, t4:t4 + 1], sc_c[t4][:],
                                            axis=AX)
                        else:
                            # last q-tile: chunk-major so each chunk's max can
                            # start while later chunks are still multiplying —
                            # this tile's softmax tail is the exposed one
                            for t4 in range(4):
                                for si, (p, kt2) in enumerate(steps):
                                    lq, lk = prods[p]
                                    nc.tensor.matmul(
                                        sc_c[t4][:],
                                        lq[:, kt2, qsl],
                                        lk[:, kt2, t4 * 512:(t4 + 1) * 512],
                                        start=(si == 0), stop=(si == 5),
                                    )
                                nc.vector.reduce_max(
                                    cmax[:, t4:t4 + 1], sc_c[t4][:], axis=AX)
                        negmax = pB.tile([128, 1], F32, tag="negmax", bufs=4,
                                         name="negmax")
                        nc.vector.reduce_max(negmax[:], cmax[:], axis=AX,
                                             negate=True)
                        attn = pB.tile([128, S], BF16, tag="attn", bufs=5, name="attn")
                        seh = pB.tile([128, 4], F32, tag="seh", bufs=4, name="seh")
                        for t4 in range(4):
                            nc.scalar.activation(
                                attn[:, t4 * 512:(t4 + 1) * 512],
                                sc_c[t4][:], AF.Exp,
                                bias=negmax[:], scale=1.0,
                                accum_out=seh[:, t4:t4 + 1],
                            )
                        nc.vector.reduce_sum(sexp[:, qt:qt + 1], seh[:], axis=AX)
                        attn_q.append(attn)
                        if pending[0] is not None:
                            pending[0][qt]()
                        if qt >= 1:
                            transposes(attn_q, attnT, qt - 1)

                    pending[0] = make_finisher(attn_q, attnT, sexp, w_out, qg)

            # tail: interleave final-projection chunks with the last
            # finisher's pieces (chunk c=0 only needs qg0's accumulator,
            # which completed during this head's qg1 scores)
            w_fin = wp.tile([128, 4, 512], F32, tag="wbig", name="w_fin")
            nc.sync.dma_start(out=w_fin[:], in_=w_fin_d[:, :, :])

            def final_chunk(c):
                for mt in range(4):
                    fp = psum.tile([128, 512], F32, tag="av", name="fp")
                    for kt in range(4):
                        nc.tensor.matmul(
                            fp[:],
                            w_fin[:, kt, mt * 128:(mt + 1) * 128],
                            acc[:, kt, c * 512:(c + 1) * 512],
                            start=(kt == 0), stop=(kt == 3),
                        )
                    outc = pB.tile([128, 512], F32, tag="outc", bufs=2,
                                   name="outc")
                    nc.scalar.copy(outc[:], fp[:])
                    nc.sync.dma_start(
                        out=outT_d[mt * 128:(mt + 1) * 128,
                                   c * 512:(c + 1) * 512],
                        in_=outc[:],
                    )

            ps_ = pending[0]
            ps_[0]()
            ps_[1]()
            final_chunk(0)
            ps_[2]()
            ps_[3]()
            final_chunk(1)


_NC_CACHE = None


def _get_nc():
    global _NC_CACHE
    if _NC_CACHE is None:
        nc = bacc.Bacc("TRN2", debug=False, num_devices=NCORES)
        with tile.TileContext(nc) as tc:
            _emit(tc)
        nc.compile()
        _NC_CACHE = nc
    return _NC_CACHE


# --------------------------------------------------------------------------
# Host-side packing
# --------------------------------------------------------------------------

def _fuse(w, blade):
    # einsum('jib,bxy->jixy', w, blade) -> mat[(i,x), (j,y)]
    wb = np.einsum("jib,bxy->jixy", w, blade)
    j, i = w.shape[0], w.shape[1]
    return np.ascontiguousarray(wb.transpose(1, 2, 0, 3)).reshape(i * 16, j * 16)


def _to_kt(m, kparts):
    # [K, F] -> [128, K//128, F] partition-major packing
    k, f = m.shape
    assert k == kparts * 128
    return np.ascontiguousarray(m.reshape(kparts, 128, f).transpose(1, 0, 2))


def _pack_weights(blade, w_enter, w_q, w_k, w_v, w_out, w_final):
    f32 = np.float32
    W_enter = _fuse(w_enter, blade).astype(f32)  # [48, 512]

    Wq = _fuse(w_q, blade)  # [512, 4096], col (j, y), j = d*8 + h
    Wq = Wq.reshape(512, 32, 8, 16)[:, :, :, INNER]   # [c, d, h, yi]
    Wq = Wq.transpose(0, 2, 1, 3).reshape(512, 8, 256)  # [c, h, (d,yi)]
    Wqs = (Wq.reshape(512, 2048) / 16.0).astype(f32)   # fold 1/sqrt(256)

    Wk = _fuse(w_k, blade)  # [512, 512], col (d, y)
    Wks = Wk.reshape(512, 32, 16)[:, :, INNER].reshape(512, 256).astype(f32)

    Wv = _fuse(w_v, blade).astype(f32)       # [512, 512]
    Wo = _fuse(w_out, blade).astype(f32)     # [4096, 512], rows (h, d, x)
    Wf = _fuse(w_final, blade).astype(f32)   # [512, 512]

    maskv = np.zeros(512, f32)
    for d in range(32):
        maskv[d * 16 + INNER] = 1.0 / 32.0
    mask = np.ascontiguousarray(maskv.reshape(4, 128).T)

    Wqs_kt = _to_kt(Wqs, 4)
    Wqsh = Wqs_kt.astype(np.float16)
    Wqsl = (Wqs_kt - Wqsh.astype(np.float32)).astype(np.float16)
    Wks_kt = _to_kt(Wks, 4)
    Wksh = Wks_kt.astype(np.float16)
    Wksl = (Wks_kt - Wksh.astype(np.float32)).astype(np.float16)
    return {
        "w_enter": W_enter,
        "w_qsh": Wqsh,
        "w_qsl": Wqsl,
        "w_ksh": Wksh,
        "w_ksl": Wksl,
        "w_v": _to_kt(Wv, 4).astype(np.float16),
        "w_out": _to_kt(Wo, 32).astype(ml_dtypes.bfloat16),
        "w_final": _to_kt(Wf, 4),
        "mask": mask,
        "ident_f": np.eye(128, dtype=f32),
        "ident_h": np.eye(128).astype(ml_dtypes.bfloat16),
    }


def kernel(x, blade, w_enter, w_q, w_k, w_v, w_out, w_final):
    global LAST_RESULTS
    x = np.asarray(x, np.float32)
    shared = _pack_weights(
        np.asarray(blade, np.float32), np.asarray(w_enter, np.float32),
        np.asarray(w_q, np.float32), np.asarray(w_k, np.float32),
        np.asarray(w_v, np.float32), np.asarray(w_out, np.float32),
        np.asarray(w_final, np.float32),
    )

    in_maps = []
    for c in range(NCORES):
        b, half = c // 2, c % 2
        xb = x[b].reshape(S, 48)
        xb = np.roll(xb, -SLAB * half, axis=0)
        m = dict(shared)
        m["xT"] = np.ascontiguousarray(xb.T)
        in_maps.append(m)

    nc = _get_nc()
    res = run_bass_kernel_spmd(
        nc, in_maps, core_ids=list(range(NCORES)), trace=TRACE,
    )
    LAST_RESULTS = res

    out = np.empty((B, S, HID, MV), np.float32)
    for c in range(NCORES):
        b, half = c // 2, c % 2
        outT = res.results[c]["outT"]  # [512, 1024]
        out[b, half * SLAB:(half + 1) * SLAB] = (
            outT.T.reshape(SLAB, HID, MV)
        )
    return out


# revision 39
# speedup vs baseline: 1.3930x; 1.0044x over previous
"""GATr-style geometric-algebra transformer block on 8 Trainium2 NeuronCores.

Strategy
--------
Host (numpy): fuse `blade` into every equi_linear weight so the device only
sees plain matmuls; pre-select the inner-product coordinates for q/k
(so qs/ks are computed directly); fold the 1/sqrt(256) attention scale into
the q weights; pre-transpose/pack everything into the exact SBUF layouts the
kernel wants.

Sharding: 8 cores = 4 batches x 2 query-halves. Every core runs the SAME
program (SPMD); the per-core "query half" is realized by rotating the token
axis of that core's input so its queries are always tokens [0, 1024).
k/v are computed over the full (rotated) sequence - softmax/attention are
permutation invariant over keys.

On-device dataflow (channel-major activations [feature_part, token_free]):
  xT -> enter proj -> hT -> equi-norm (masked mean-square via mask-matmul,
  rsqrt, gpsimd partition-broadcast) -> hn -> ks/v projections
  -> per head: qs proj, scores = qsT.T @ ksT (PSUM, fp32),
     softmax (DVE row-max + ACT exp with per-partition bias + accum sum),
     PE-transpose of bf16 probs -> attnT, attn@v in bf16 (fp32 PSUM accum),
     1/sumexp folded into the PSUM->SBUF copy of the per-head output,
     out_proj accumulated into acc (which was initialized with the residual),
  -> final projection -> outT.
"""

import sys

import numpy as np

for _p in ("/opt/trn_rl_repo", "/root/.axon_site/_ro/trn_rl_repo"):
    if _p not in sys.path:
        sys.path.insert(0, _p)

import ml_dtypes  # noqa: E402

import concourse.bacc as bacc  # noqa: E402
import concourse.tile as tile  # noqa: E402
from concourse import mybir  # noqa: E402
from concourse.bass_utils import run_bass_kernel_spmd  # noqa: E402

F32 = mybir.dt.float32
F16 = mybir.dt.float16
BF16 = mybir.dt.bfloat16
AX = mybir.AxisListType.X
AF = mybir.ActivationFunctionType

INNER = np.array([0, 2, 3, 4, 8, 9, 10, 14])
B, S, C_IN, MV = 4, 2048, 3, 16
HID, NH = 32, 8
NCORES = 8
SLAB = S // 2  # queries per core

# Set by test.py to collect an NTFF profile.
TRACE = False
LAST_RESULTS = None


# --------------------------------------------------------------------------
# Device program
# --------------------------------------------------------------------------

def _emit(tc):
    nc = tc.nc

    xT_d = nc.declare_dram_parameter("xT", [48, S], F32, isOutput=False)
    w_enter_d = nc.declare_dram_parameter("w_enter", [48, 512], F32, isOutput=False)
    w_qsh_d = nc.declare_dram_parameter("w_qsh", [128, 4, 2048], F16, isOutput=False)
    w_qsl_d = nc.declare_dram_parameter("w_qsl", [128, 4, 2048], F16, isOutput=False)
    w_ksh_d = nc.declare_dram_parameter("w_ksh", [128, 4, 256], F16, isOutput=False)
    w_ksl_d = nc.declare_dram_parameter("w_ksl", [128, 4, 256], F16, isOutput=False)
    w_v_d = nc.declare_dram_parameter("w_v", [128, 4, 512], F16, isOutput=False)
    w_out_d = nc.declare_dram_parameter("w_out", [128, 32, 512], BF16, isOutput=False)
    w_fin_d = nc.declare_dram_parameter("w_final", [128, 4, 512], F32, isOutput=False)
    mask_d = nc.declare_dram_parameter("mask", [128, 4], F32, isOutput=False)
    idf_d = nc.declare_dram_parameter("ident_f", [128, 128], F32, isOutput=False)
    idh_d = nc.declare_dram_parameter("ident_h", [128, 128], BF16, isOutput=False)
    outT_d = nc.declare_dram_parameter("outT", [512, SLAB], F32, isOutput=True)

    from contextlib import ExitStack

    with ExitStack() as ctx:
        psum = ctx.enter_context(tc.tile_pool(name="ps", bufs=1, space="PSUM"))
        pp = ctx.enter_context(tc.tile_pool(name="persist", bufs=1))
        wp = ctx.enter_context(tc.tile_pool(name="wbig", bufs=2))

        # ---- persistent tiles -------------------------------------------
        # hn kept only as fp16 hi/lo splits (+ bf16 for the v path):
        # every consumer matmul runs split-fp16 or bf16.
        hh = pp.tile([128, 4, S], F16, name="hh")
        hl = pp.tile([128, 4, S], F16, name="hl")
        # ks stored as fp16 hi/lo split (scores run as 3 fp16 products)
        kh = pp.tile([128, 2, S], F16, name="kh")
        kl = pp.tile([128, 2, S], F16, name="kl")
        v_tok = pp.tile([128, 16, 512], BF16, name="v_tok")
        acc = pp.tile([128, 4, SLAB], F32, name="acc")
        w_ksh = pp.tile([128, 4, 256], F16, name="w_ksh")
        w_ksl = pp.tile([128, 4, 256], F16, name="w_ksl")
        mask = pp.tile([128, 4], F32, name="mask")
        idf = pp.tile([128, 128], F32, name="idf")
        idh = pp.tile([128, 128], BF16, name="idh")

        nc.sync.dma_start(out=w_ksh[:], in_=w_ksh_d[:, :, :])
        nc.sync.dma_start(out=w_ksl[:], in_=w_ksl_d[:, :, :])
        nc.sync.dma_start(out=mask[:], in_=mask_d[:, :])
        nc.sync.dma_start(out=idf[:], in_=idf_d[:, :])
        nc.sync.dma_start(out=idh[:], in_=idh_d[:, :])

        # ================= phase A: enter, norm, k/v =====================
        with tc.tile_pool(name="pA", bufs=1) as pA:
            xT = pA.tile([48, S], F32, name="xT")
            w_enter = pA.tile([48, 512], F32, name="w_enter")
            hT = pA.tile([128, 4, S], F32, name="hT")
            nc.sync.dma_start(out=xT[:], in_=xT_d[:, :])
            nc.sync.dma_start(out=w_enter[:], in_=w_enter_d[:, :])

            # enter projection: hT[mt] = W_enter[:, mt].T @ xT
            for mt in range(4):
                for c in range(4):
                    ps = psum.tile([128, 512], F32, tag="av", name="ps_ent")
                    nc.tensor.matmul(
                        ps[:],
                        w_enter[0:48, mt * 128:(mt + 1) * 128],
                        xT[0:48, c * 512:(c + 1) * 512],
                        start=True, stop=True,
                    )
                    nc.scalar.copy(hT[:, mt, c * 512:(c + 1) * 512], ps[:])
                # residual lives in acc
                nc.scalar.copy(acc[:, mt, :], hT[:, mt, 0:SLAB])

            # equi-norm: ip = masked mean of squares over channels (PE reduce)
            ipc = [psum.tile([1, 512], F32, tag="sc", bufs=4, name="ipc")
                   for _ in range(4)]
            for c in range(4):
                for kt in range(4):
                    hsq = pA.tile([128, 512], F32, tag="hsq", bufs=3,
                                  name="hsq")
                    nc.scalar.activation(hsq[:], hT[:, kt,
                                         c * 512:(c + 1) * 512], AF.Square)
                    nc.tensor.matmul(
                        ipc[c][0:1, :],
                        mask[:, kt:kt + 1],
                        hsq[:],
                        start=(kt == 0), stop=(kt == 3),
                    )
            sq_s = pA.tile([1, S], F32, name="sq_s")
            r_tok = pA.tile([1, S], F32, name="r_tok")
            rb = pA.tile([128, S], F32, name="rb")
            for c in range(4):
                cs = slice(c * 512, (c + 1) * 512)
                nc.scalar.activation(sq_s[0:1, cs], ipc[c][0:1, :], AF.Sqrt)
                nc.vector.reciprocal(r_tok[0:1, cs], sq_s[0:1, cs])
                nc.gpsimd.partition_broadcast(rb[:, cs], r_tok[0:1, cs])
                for kt in range(4):
                    hnc = pA.tile([128, 512], F32, tag="hnc", bufs=3,
                                  name="hnc")
                    nc.vector.tensor_mul(hnc[:], hT[:, kt, cs], rb[:, cs])
                    nc.scalar.copy(hh[:, kt, cs], hnc[:])
                    nc.vector.tensor_sub(hl[:, kt, cs], hnc[:], hh[:, kt, cs])

            # ks projection (3-term fp16 split products, fp32 PSUM)
            kprods = [(w_ksh, hh), (w_ksl, hh), (w_ksh, hl)]
            for mt in range(2):
                for c in range(4):
                    ps = psum.tile([128, 512], F32, tag="av", name="ps_ks")
                    for p in range(3):
                        lw, lh = kprods[p]
                        for kt in range(4):
                            nc.tensor.matmul(
                                ps[:],
                                lw[:, kt, mt * 128:(mt + 1) * 128],
                                lh[:, kt, c * 512:(c + 1) * 512],
                                start=(p == 0 and kt == 0),
                                stop=(p == 2 and kt == 3),
                            )
                    nc.scalar.copy(kh[:, mt, c * 512:(c + 1) * 512], ps[:])
                    nc.vector.tensor_sub(kl[:, mt, c * 512:(c + 1) * 512],
                                         ps[:], kh[:, mt, c * 512:(c + 1) * 512])

            # v projection weights (the matmuls run as initial pipeline
            # pieces interleaved with the first head's softmax chains)
            w_v = wp.tile([128, 4, 512], F16, tag="wbig", name="w_v")
            nc.sync.dma_start(out=w_v[:], in_=w_v_d[:, :, :])

        def v_piece(j):
            def piece():
                for tt in range(4 * j, 4 * j + 4):
                    ps = psum.tile([128, 512], F32, tag="av", name="ps_v")
                    for kt in range(4):
                        nc.tensor.matmul(
                            ps[:],
                            hh[:, kt, tt * 128:(tt + 1) * 128],
                            w_v[:, kt, :],
                            start=(kt == 0), stop=(kt == 3),
                        )
                    nc.scalar.copy(v_tok[:, tt, :], ps[:])
            return piece

        # ================= phase B: attention over 8 heads ===============
        # Software pipeline: for each (h, qg), the tail stages (last q-tile's
        # transposes, attn@v, out_proj) are deferred until after the NEXT
        # (h, qg)'s scores+softmax have been emitted, so the exp latency of
        # the last q-tile always hides under independent PE work.
        with tc.tile_pool(name="pB", bufs=1) as pB:
            pending = [[v_piece(j) for j in range(4)]]

            def transposes(attn_q, attnT, qt):
                # qt-major: 16 transposed blocks of attn[qt] -> attnT
                for j in range(4):
                    tr = psum.tile([128, 512], BF16, tag="tr", name="tr")
                    for k in range(4):
                        tt = j * 4 + k
                        nc.tensor.transpose(
                            tr[:, k * 128:(k + 1) * 128],
                            attn_q[qt][:, tt * 128:(tt + 1) * 128],
                            idh[:],
                        )
                    nc.scalar.copy(
                        attnT[j][:, :, qt * 128:(qt + 1) * 128],
                        tr[:].rearrange("p (k q) -> p k q", k=4),
                    )

            def make_finisher(attn_q, attnT, sexp, w_out, qg):
                # Four pieces, fired one per q-tile of the NEXT scores block,
                # so PE always has independent work while softmax chains run.
                state = {}

                def p0():
                    transposes(attn_q, attnT, 3)
                    # 1/sumexp -> free axis -> broadcast over partitions
                    st = psum.tile([1, 512], F32, tag="tr", name="st")
                    for qt in range(4):
                        nc.tensor.transpose(
                            st[0:1, qt * 128:(qt + 1) * 128],
                            sexp[:, qt:qt + 1], idf[:],
                        )
                    rqT = pB.tile([1, 512], F32, tag="rqT", bufs=2, name="rqT")
                    nc.vector.reciprocal(rqT[:], st[0:1, :])
                    rb_q = pB.tile([128, 512], F32, tag="rbq", bufs=2, name="rb_q")
                    nc.gpsimd.partition_broadcast(rb_q[:], rqT[0:1, :])
                    state["rb_q"] = rb_q
                    state["oT"] = pB.tile([128, 4, 512], BF16, tag="oT",
                                          bufs=2, name="oT")

                def av(mt):
                    avp = psum.tile([128, 512], F32, tag="av", name="avp")
                    for tt in range(16):
                        nc.tensor.matmul(
                            avp[:],
                            v_tok[:, tt, mt * 128:(mt + 1) * 128],
                            attnT[tt // 4][:, tt % 4, :],
                            start=(tt == 0), stop=(tt == 15),
                        )
                    nc.vector.tensor_mul(state["oT"][:, mt, :], avp[:],
                                         state["rb_q"][:])

                def p1():
                    av(0)
                    av(1)

                def p2():
                    av(2)
                    av(3)

                def p3():
                    for mt2 in range(4):
                        opp = psum.tile([128, 512], F32, tag="av", name="opp")
                        for kt in range(4):
                            nc.tensor.matmul(
                                opp[:],
                                w_out[:, kt, mt2 * 128:(mt2 + 1) * 128],
                                state["oT"][:, kt, :],
                                start=(kt == 0), stop=(kt == 3),
                            )
                        sl = acc[:, mt2, qg * 512:(qg + 1) * 512]
                        nc.vector.tensor_add(sl, sl, opp[:])
                return [p0, p1, p2, p3]

            for h in range(NH):
                w_qsh = pB.tile([128, 4, 256], F16, tag="wqsh", bufs=2,
                                name="w_qsh")
                w_qsl = pB.tile([128, 4, 256], F16, tag="wqsl", bufs=2,
                                name="w_qsl")
                nc.sync.dma_start(out=w_qsh[:],
                                  in_=w_qsh_d[:, :, h * 256:(h + 1) * 256])
                nc.sync.dma_start(out=w_qsl[:],
                                  in_=w_qsl_d[:, :, h * 256:(h + 1) * 256])
                w_out = wp.tile([128, 4, 512], BF16, tag="wbig", name="w_out")
                nc.sync.dma_start(out=w_out[:], in_=w_out_d[:, 4 * h:4 * h + 4, :])

                # qs projection (3-term fp16 split products, fp32 PSUM)
                qh = pB.tile([128, 2, SLAB], F16, tag="qh", bufs=2, name="qh")
                ql = pB.tile([128, 2, SLAB], F16, tag="ql", bufs=2, name="ql")
                qprods = [(w_qsh, hh), (w_qsl, hh), (w_qsh, hl)]
                for mt in range(2):
                    for c in range(SLAB // 512):
                        ps = psum.tile([128, 512], F32, tag="tr", name="ps_qs")
                        for p in range(3):
                            lw, lh = qprods[p]
                            for kt in range(4):
                                nc.tensor.matmul(
                                    ps[:],
                                    lw[:, kt, mt * 128:(mt + 1) * 128],
                                    lh[:, kt, c * 512:(c + 1) * 512],
                                    start=(p == 0 and kt == 0),
                                    stop=(p == 2 and kt == 3),
                                )
                        cs = slice(c * 512, (c + 1) * 512)
                        nc.scalar.copy(qh[:, mt, cs], ps[:])
                        nc.vector.tensor_sub(ql[:, mt, cs], ps[:], qh[:, mt, cs])

                for qg in range(SLAB // 512):
                    attn_q = []
                    sexp = pB.tile([128, 4], F32, tag="sexp", bufs=2, name="sexp")
                    attnT = []
                    for j in range(4):
                        attnT.append(pB.tile([128, 4, 512], BF16, tag="attnT",
                                             bufs=8, name="attnT"))

                    for qt in range(4):
                        qtg = qg * 4 + qt
                        qsl = slice(qtg * 128, (qtg + 1) * 128)
                        sc_c = [psum.tile([128, 512], F32, tag="sc", bufs=4,
                                          name="scc") for _ in range(4)]
                        cmax = pB.tile([128, 4], F32, tag="cmax", bufs=4,
                                       name="cmax")
                        # scores = qh*kh + ql*kh + qh*kl  (fp16 split, fp32 acc)
                        prods = [(qh, kh), (ql, kh), (qh, kl)]
                        steps = [(p, kt2) for p in range(3) for kt2 in range(2)]
                        if qt < 3:
                            # step-major: stationary operand reused across chunks
                            for si, (p, kt2) in enumerate(steps):
                                lq, lk = prods[p]
                                for t4 in range(4):
                                    nc.tensor.matmul(
                                        sc_c[t4][:],
                                        lq[:, kt2, qsl],
                                        lk[:, kt2, t4 * 512:(t4 + 1) * 512],
                                        start=(si == 0), stop=(si == 5),
                                    )
                                    if si == 5:
                                        nc.vector.reduce_max(
                                            cmax[:, t4:t4 + 1], sc_c[t4][:],
                                            axis=AX)
                        else:
                            # last q-tile: chunk-major so each chunk's max can
                            # start while later chunks are still multiplying —
                            # this tile's softmax tail is the exposed one
                            for t4 in range(4):
                                for si, (p, kt2) in enumerate(steps):
                                    lq, lk = prods[p]
                                    nc.tensor.matmul(
                                        sc_c[t4][:],
                                        lq[:, kt2, qsl],
                                        lk[:, kt2, t4 * 512:(t4 + 1) * 512],
                                        start=(si == 0), stop=(si == 5),
                                    )
                                nc.vector.reduce_max(
                                    cmax[:, t4:t4 + 1], sc_c[t4][:], axis=AX)
                        negmax = pB.tile([128, 1], F32, tag="negmax", bufs=4,
                                         name="negmax")
                        nc.vector.reduce_max(negmax[:], cmax[:], axis=AX,
                                             negate=True)
                        attn = pB.tile([128, S], BF16, tag="attn", bufs=5, name="attn")
                        seh = pB.tile([128, 4], F32, tag="seh", bufs=4, name="seh")
                        for t4 in range(4):
                            nc.scalar.activation(
                                attn[:, t4 * 512:(t4 + 1) * 512],
                                sc_c[t4][:], AF.Exp,
                                bias=negmax[:], scale=1.0,
                                accum_out=seh[:, t4:t4 + 1],
                            )
                        nc.vector.reduce_sum(sexp[:, qt:qt + 1], seh[:], axis=AX)
                        attn_q.append(attn)
                        if pending[0] is not None:
                            pending[0][qt]()
                        if qt >= 1:
                            transposes(attn_q, attnT, qt - 1)

                    pending[0] = make_finisher(attn_q, attnT, sexp, w_out, qg)

            # tail: interleave final-projection chunks with the last
            # finisher's pieces (chunk c=0 only needs qg0's accumulator,
            # which completed during this head's qg1 scores)
            w_fin = wp.tile([128, 4, 512], F32, tag="wbig", name="w_fin")
            nc.sync.dma_start(out=w_fin[:], in_=w_fin_d[:, :, :])

            def final_chunk(c):
                for mt in range(4):
                    fp = psum.tile([128, 512], F32, tag="av", name="fp")
                    for kt in range(4):
                        nc.tensor.matmul(
                            fp[:],
                            w_fin[:, kt, mt * 128:(mt + 1) * 128],
                            acc[:, kt, c * 512:(c + 1) * 512],
                            start=(kt == 0), stop=(kt == 3),
                        )
                    outc = pB.tile([128, 512], F32, tag="outc", bufs=2,
                                   name="outc")
                    nc.scalar.copy(outc[:], fp[:])
                    nc.sync.dma_start(
                        out=outT_d[mt * 128:(mt + 1) * 128,
                                   c * 512:(c + 1) * 512],
                        in_=outc[:],
                    )

            ps_ = pending[0]
            ps_[0]()
            ps_[1]()
            final_chunk(0)
            ps_[2]()
            ps_[3]()
            final_chunk(1)


_NC_CACHE = None


def _get_nc():
    global _NC_CACHE
    if _NC_CACHE is None:
        nc = bacc.Bacc("TRN2", debug=False, num_devices=NCORES)
        with tile.TileContext(nc) as tc:
            _emit(tc)
        nc.compile()
        _NC_CACHE = nc
    return _NC_CACHE


# --------------------------------------------------------------------------
# Host-side packing
# --------------------------------------------------------------------------

def _fuse(w, blade):
    # einsum('jib,bxy->jixy', w, blade) -> mat[(i,x), (j,y)]
    wb = np.einsum("jib,bxy->jixy", w, blade)
    j, i = w.shape[0], w.shape[1]
    return np.ascontiguousarray(wb.transpose(1, 2, 0, 3)).reshape(i * 16, j * 16)


def _to_kt(m, kparts):
    # [K, F] -> [128, K//128, F] partition-major packing
    k, f = m.shape
    assert k == kparts * 128
    return np.ascontiguousarray(m.reshape(kparts, 128, f).transpose(1, 0, 2))


def _pack_weights(blade, w_enter, w_q, w_k, w_v, w_out, w_final):
    f32 = np.float32
    W_enter = _fuse(w_enter, blade).astype(f32)  # [48, 512]

    Wq = _fuse(w_q, blade)  # [512, 4096], col (j, y), j = d*8 + h
    Wq = Wq.reshape(512, 32, 8, 16)[:, :, :, INNER]   # [c, d, h, yi]
    Wq = Wq.transpose(0, 2, 1, 3).reshape(512, 8, 256)  # [c, h, (d,yi)]
    Wqs = (Wq.reshape(512, 2048) / 16.0).astype(f32)   # fold 1/sqrt(256)

    Wk = _fuse(w_k, blade)  # [512, 512], col (d, y)
    Wks = Wk.reshape(512, 32, 16)[:, :, INNER].reshape(512, 256).astype(f32)

    Wv = _fuse(w_v, blade).astype(f32)       # [512, 512]
    Wo = _fuse(w_out, blade).astype(f32)     # [4096, 512], rows (h, d, x)
    Wf = _fuse(w_final, blade).astype(f32)   # [512, 512]

    maskv = np.zeros(512, f32)
    for d in range(32):
        maskv[d * 16 + INNER] = 1.0 / 32.0
    mask = np.ascontiguousarray(maskv.reshape(4, 128).T)

    Wqs_kt = _to_kt(Wqs, 4)
    Wqsh = Wqs_kt.astype(np.float16)
    Wqsl = (Wqs_kt - Wqsh.astype(np.float32)).astype(np.float16)
    Wks_kt = _to_kt(Wks, 4)
    Wksh = Wks_kt.astype(np.float16)
    Wksl = (Wks_kt - Wksh.astype(np.float32)).astype(np.float16)
    return {
        "w_enter": W_enter,
        "w_qsh": Wqsh,
        "w_qsl": Wqsl,
        "w_ksh": Wksh,
        "w_ksl": Wksl,
        "w_v": _to_kt(Wv, 4).astype(np.float16),
        "w_out": _to_kt(Wo, 32).astype(ml_dtypes.bfloat16),
        "w_final": _to_kt(Wf, 4),
        "mask": mask,
        "ident_f": np.eye(128, dtype=f32),
        "ident_h": np.eye(128).astype(ml_dtypes.bfloat16),
    }


def kernel(x, blade, w_enter, w_q, w_k, w_v, w_out, w_final):
    global LAST_RESULTS
    x = np.asarray(x, np.float32)
    shared = _pack_weights(
        np.asarray(blade, np.float32), np.asarray(w_enter, np.float32),
        np.asarray(w_q, np.float32), np.asarray(w_k, np.float32),
        np.asarray(w_v, np.float32), np.asarray(w_out, np.float32),
        np.asarray(w_final, np.float32),
    )

    in_maps = []
    for c in range(NCORES):
        b, half = c // 2, c % 2
        xb = x[b].reshape(S, 48)
        xb = np.roll(xb, -SLAB * half, axis=0)
        m = dict(shared)
        m["xT"] = np.ascontiguousarray(xb.T)
        in_maps.append(m)

    nc = _get_nc()
    res = run_bass_kernel_spmd(
        nc, in_maps, core_ids=list(range(NCORES)), trace=TRACE,
    )
    LAST_RESULTS = res

    out = np.empty((B, S, HID, MV), np.float32)
    for c in range(NCORES):
        b, half = c // 2, c % 2
        outT = res.results[c]["outT"]  # [512, 1024]
        out[b, half * SLAB:(half + 1) * SLAB] = (
            outT.T.reshape(SLAB, HID, MV)
        )
    return out
